# revision 1
# baseline (speedup 1.0000x reference)
"""MLA (absorbed-weight multi-head latent attention) TRN2 Bass kernel.

Problem: B=2, N=NKV=2048, E=4096, H=16, HD=256, LQ=512, LKV=256.
  C_q  = Q @ Wq_d                 [B,N,LQ]
  C_kv = K @ Wkv_d                [B,Nkv,LKV]
  CqWqk = (C_q @ W_qk)            [B,N,H,LKV]
  scores = einsum('bnhl,bkl->bhnk', CqWqk, C_kv) / sqrt(LKV)
  attn = softmax(scores, -1)
  V_up = (C_kv @ Wv_u)            [B,Nkv,H,HD]
  out  = einsum('bhnk,bkhd->bnhd', attn, V_up) -> [B,N,E]

Sharding: 8 cores = (batch b in 0..1) x (query quarter q in 0..3).
Each core handles n-rows [q*512,(q+1)*512) of batch b for ALL heads.

All matmuls run in float32r (TF32-like, ~2^-11 rel rounding, full PE rate).
Host passes Q^T and K^T slices so the device needs no transposes:
  C_qT   [LQ, n]  = lhsT Wq_d   @ rhs Q^T      (contract E)
  C_kvT  [LKV, k] = lhsT Wkv_d  @ rhs K^T      (contract E)
  CqWqkT [LKV, n] = lhsT W_qk_h @ rhs C_qT     (contract LQ)
  Vup_h  [k, HD]  = lhsT C_kvT  @ rhs Wv_u_h   (contract LKV)
  S^T    [k, n]   = lhsT C_kvT  @ rhs CqWqkT_h (contract LKV)
  P^T    = exp(S^T / 16)  (no max-subtraction: |S| <= ~6, fp32-safe)
  out    [n, HD+] = lhsT P^T    @ rhs [Vup_h | 1 1]  (contract k)
  out[:, :256] /= out[:, 256]  (ones-column row-sum denominator)
"""
import numpy as np

B, N, NKV, E, H = 2, 2048, 2048, 4096, 16
HD, LQ, LKV = 256, 512, 256
NCORES = 8
NQ = N // 4          # 512 query rows per core
ECH = E // 128       # 32 e-chunks
KCH = NKV // 128     # 16 k-chunks
NCK = NQ // 128      # 4 n-chunks per core

_cache = {}


def build_nc(iters=1, stop_after="full", fake_ckv=False):
    import concourse.bass as bass
    from concourse import bacc
    import concourse.mybir as mybir
    import concourse.tile as tile

    dt = mybir.dt
    f32r = dt.float32r
    f32 = dt.float32
    do_proj = stop_after in ("proj", "scores", "full")
    do_headmm = stop_after in ("scores", "full")
    do_scores = stop_after == "full"

    nc = bacc.Bacc(None, target_bir_lowering=False)
    QT = nc.dram_tensor("QT", [E, NQ], f32r, kind="ExternalInput")
    KT = nc.dram_tensor("KT", [E, NKV], f32r, kind="ExternalInput")
    WQD = nc.dram_tensor("WQD", [E, LQ], f32r, kind="ExternalInput")
    WQK = nc.dram_tensor("WQK", [LQ, H * LKV], f32r, kind="ExternalInput")
    WKVD = nc.dram_tensor("WKVD", [E, LKV], f32r, kind="ExternalInput")
    WVU = nc.dram_tensor("WVU", [LKV, H * HD], f32r, kind="ExternalInput")
    ONES = nc.dram_tensor("ONES", [128, 32], f32r, kind="ExternalInput")
    OUT = nc.dram_tensor("OUT", [NQ, E], f32, kind="ExternalOutput")

    Exp = mybir.ActivationFunctionType.Exp

    with tile.TileContext(nc) as tc:
        with tc.tile_pool(name="persist", bufs=1) as persist, \
             tc.tile_pool(name="psumA", bufs=1, space="PSUM") as psA, \
             tc.tile_pool(name="psumB", bufs=2, space="PSUM") as psB:
            loop_ctx = tc.For_i(0, iters, 1,
                                hint_engines=(mybir.EngineType.PE,)) \
                if iters > 1 else None
            if loop_ctx is not None:
                loop_ctx.__enter__()

            cqt = persist.tile([128, 4, NQ], f32r)       # C_qT  [LQ, n]
            ckvt = persist.tile([128, 2, NKV], f32r)     # C_kvT [LKV, k]

            # ---------- phase 1: C_qT (sliced loads), phase 2: C_kvT ----------
            with tc.tile_pool(name="ph1", bufs=1) as ph1, \
                 tc.tile_pool(name="ktp", bufs=3) as ktp:
                qt = ph1.tile([128, ECH, NQ], f32r)
                wqd = ph1.tile([128, ECH, LQ], f32r)
                nc.sync.dma_start(out=qt, in_=QT.rearrange("(c p) n -> p c n", p=128))
                nc.sync.dma_start(out=wqd, in_=WQD.rearrange("(c p) l -> p c l", p=128))
                if do_proj:
                    # C_qT: 4 x [128, 512], contract E
                    for lc in range(4):
                        ps = psB.tile([128, 512], f32, tag="sw")
                        for ec in range(ECH):
                            nc.tensor.matmul(ps, wqd[:, ec, lc * 128:(lc + 1) * 128],
                                             qt[:, ec, :],
                                             start=(ec == 0), stop=(ec == ECH - 1))
                        nc.vector.tensor_copy(cqt[:, lc, :], ps)
                else:
                    nc.vector.tensor_copy(cqt[:, 0, :], qt[:, 0, :])

                # C_kvT: 8 accumulators [2 lkc x 4 ktile] over streamed KT
                if fake_ckv:
                    nc.sync.dma_start(
                        out=ckvt,
                        in_=KT[0:256, :].rearrange("(l p) n -> p l n", p=128))
                if do_proj and not fake_ckv:
                    accs = [psA.tile([128, 512], f32, tag=t, name=f"acc_{t}")
                            for t in ("o0", "o1", "o2", "o3")]
                    accs += [psB.tile([128, 512], f32, tag=t, name=f"acc2_{i}")
                             for i, t in enumerate(("v", "v", "sw", "sw"))]
                for ec in range(0 if not fake_ckv else ECH, ECH):
                    ktt = ktp.tile([128, NKV], f32r, tag="kt")
                    nc.sync.dma_start(out=ktt, in_=KT[ec * 128:(ec + 1) * 128, :])
                    if ec % 4 == 0:
                        wkvd_t = ktp.tile([128, 4, LKV], f32r, tag="wkvd")
                        nc.sync.dma_start(
                            out=wkvd_t,
                            in_=WKVD[ec * 128:(ec + 4) * 128, :]
                            .rearrange("(c p) l -> p c l", p=128))
                    if do_proj:
                        for lc in range(2):
                            for nt in range(4):
                                nc.tensor.matmul(
                                    accs[lc * 4 + nt],
                                    wkvd_t[:, ec % 4, lc * 128:(lc + 1) * 128],
                                    ktt[:, nt * 512:(nt + 1) * 512],
                                    start=(ec == 0), stop=(ec == ECH - 1))
                    else:
                        if ec == 0:
                            nc.vector.tensor_copy(ckvt[:, 0, 0:NKV], ktt)
                if do_proj and not fake_ckv:
                    for lc in range(2):
                        for nt in range(4):
                            dst = ckvt[:, lc, nt * 512:(nt + 1) * 512]
                            if nt % 2 == 0:
                                nc.vector.tensor_copy(dst, accs[lc * 4 + nt])
                            else:
                                nc.scalar.copy(dst, accs[lc * 4 + nt])

            # ---------- phase 3: per-head attention ----------
            with tc.tile_pool(name="head", bufs=2) as hp, \
                 tc.tile_pool(name="ptp", bufs=3) as ptp:
                for h in range(H):
                    wqk_h = hp.tile([128, 4, LKV], f32r, tag="wqk")
                    nc.sync.dma_start(
                        out=wqk_h,
                        in_=WQK[:, h * LKV:(h + 1) * LKV]
                        .rearrange("(c p) l -> p c l", p=128))
                    wvu_h = hp.tile([128, 2, HD], f32r, tag="wvu")
                    nc.sync.dma_start(
                        out=wvu_h,
                        in_=WVU[:, h * HD:(h + 1) * HD]
                        .rearrange("(c p) d -> p c d", p=128))

                    if do_headmm:
                        vup = hp.tile([128, KCH, 258], f32r, tag="vup")
                        nc.sync.dma_start(
                            out=vup[:, :, 256:258],
                            in_=ONES.rearrange("p (g c) -> p g c", c=2))
                        # CqWqkT_h [2 x 128, n=512], contract LQ
                        cqwqk = hp.tile([128, 2, NQ], f32r, tag="cqwqk")
                        for lkc in range(2):
                            ps = psB.tile([128, 512], f32, tag="sw")
                            for lc in range(4):
                                nc.tensor.matmul(
                                    ps, wqk_h[:, lc, lkc * 128:(lkc + 1) * 128],
                                    cqt[:, lc, :], start=(lc == 0), stop=(lc == 3))
                            nc.scalar.copy(cqwqk[:, lkc, :], ps)

                        # V_up rows for this head, [k, 256] per k-chunk
                        for kc in range(KCH):
                            psv = psB.tile([128, 256], f32, tag="v")
                            for lkc in range(2):
                                nc.tensor.matmul(
                                    psv, ckvt[:, lkc, kc * 128:(kc + 1) * 128],
                                    wvu_h[:, lkc, :],
                                    start=(lkc == 0), stop=(lkc == 1))
                            nc.vector.tensor_copy(vup[:, kc, 0:256], psv)

                    if do_scores:
                        # scores^T -> exp -> PV accumulate
                        pso = [psA.tile([128, 258], f32, tag=f"o{i}",
                                        name=f"pso{i}") for i in range(NCK)]
                        for kc in range(KCH):
                            pss = psB.tile([128, 512], f32, tag="sw")
                            for lkc in range(2):
                                nc.tensor.matmul(
                                    pss, ckvt[:, lkc, kc * 128:(kc + 1) * 128],
                                    cqwqk[:, lkc, :],
                                    start=(lkc == 0), stop=(lkc == 1))
                            pt = ptp.tile([128, NQ], f32r, tag="pt")
                            nc.scalar.activation(out=pt, in_=pss, func=Exp,
                                                 scale=1.0 / 16.0)
                            for nk in range(NCK):
                                nc.tensor.matmul(
                                    pso[nk], pt[:, nk * 128:(nk + 1) * 128],
                                    vup[:, kc, :],
                                    start=(kc == 0), stop=(kc == KCH - 1))

                        # normalize + store (one batched DMA per head)
                        ot = ptp.tile([128, NCK, HD], f32, tag="ot")
                        for nk in range(NCK):
                            den = hp.tile([128, 1], f32, tag="den")
                            nc.vector.reciprocal(den, pso[nk][:, 256:257])
                            nc.vector.tensor_scalar_mul(ot[:, nk, :],
                                                        pso[nk][:, 0:256], den)
                        nc.sync.dma_start(
                            out=OUT.rearrange("(c p) e -> p c e", p=128)
                            [:, :, h * HD:(h + 1) * HD],
                            in_=ot)
                    else:
                        dummy = ptp.tile([128, NCK, HD], f32, tag="ot")
                        nc.vector.memset(dummy, 0.5)
                        nc.sync.dma_start(
                            out=OUT.rearrange("(c p) e -> p c e", p=128)
                            [:, :, h * HD:(h + 1) * HD],
                            in_=dummy)

            if loop_ctx is not None:
                loop_ctx.__exit__(None, None, None)

    nc.finalize()
    return nc


def get_nc(iters=1, stop_after="full", fake_ckv=False):
    key = (iters, stop_after, fake_ckv)
    if key not in _cache:
        _cache[key] = build_nc(iters, stop_after, fake_ckv)
    return _cache[key]


def make_in_maps(Q, K, Wq_d, W_qk, Wkv_d, Wv_u):
    Q = np.asarray(Q, dtype=np.float32)
    K = np.asarray(K, dtype=np.float32)
    ones = np.ones((128, 32), dtype=np.float32)
    weights = {
        "WQD": np.ascontiguousarray(np.asarray(Wq_d, dtype=np.float32)),
        "WQK": np.ascontiguousarray(np.asarray(W_qk, dtype=np.float32)),
        "WKVD": np.ascontiguousarray(np.asarray(Wkv_d, dtype=np.float32)),
        "WVU": np.ascontiguousarray(np.asarray(Wv_u, dtype=np.float32)),
        "ONES": ones,
    }
    kts = [np.ascontiguousarray(K[b].T) for b in range(B)]
    qts = [np.ascontiguousarray(Q[b].T) for b in range(B)]
    in_maps = []
    for c in range(NCORES):
        b, q = divmod(c, 4)
        m = dict(weights)
        m["KT"] = kts[b]
        m["QT"] = np.ascontiguousarray(qts[b][:, q * NQ:(q + 1) * NQ])
        in_maps.append(m)
    return in_maps


def kernel(Q, K, Wq_d, W_qk, Wkv_d, Wv_u):
    from concourse.bass_utils import run_bass_kernel_spmd

    nc = get_nc(1)
    in_maps = make_in_maps(Q, K, Wq_d, W_qk, Wkv_d, Wv_u)
    res = run_bass_kernel_spmd(nc, in_maps, core_ids=list(range(NCORES)))
    out = np.empty((B, N, E), dtype=np.float32)
    for c in range(NCORES):
        b, q = divmod(c, 4)
        out[b, q * NQ:(q + 1) * NQ, :] = res.results[c]["OUT"]
    return out

